# revision 1
# baseline (speedup 1.0000x reference)
"""AAA-protected classifier kernel for Trainium2, 8 NeuronCores.

Math (see reference): z = x@W+b; y = argmax(z); 100 AdamW steps on u (init z)
minimizing  sum_i |ce(u_i,y_i) - l_trg_i| + BETA*|max_all(softmax(u)) - p_trg_i|.

Gradient (faithful to jax autodiff), with e = exp(u), s = rowsum e, p = e/s:
  G = sign1_i * (p - onehot_y) + C1 * ind_i * (onehot_jmax - p)
  sign1_i = sign(ce_i - l_trg_i);  s_pmax = global max p (one AllGather/iter)
  S2 = sum_i sign(s_pmax - p_trg_i)  (from the once-gathered p_trg vector)
  C1 = BETA * S2 * s_pmax;  ind_i = row i attains the global max.
Fused form used below (y-column lives in side state, onehot_y term separate):
  gIn = (1-B1)*G = e (x) rw_i  +  mask (x) b1_i
  rw_i = (1-B1)*rs_i*(sign1_i - C1*ind_i),  b1_i = (1-B1)*C1*ind_i
  mask = is_equal(u, rmu'_i)  (rmu' = rowmax u, +1e30 when y col is the argmax)

Sharding: batch 4096 -> 512 rows/core (data parallel), W/b replicated.
Layout per core: [128 partitions, 4 segments, 1000 classes]; row = s*128+p.

The y-column of each row is poisoned to -1e4 in the main tensor (exp -> 0,
never wins a max, gradient stays exactly 0) and tracked exactly in [128,4]
side state (u_y, m_y, v_y), merged back at the end.
"""

import math
import numpy as np

import bass_rust as _bass_rust
import concourse.bass as bass
import concourse.bacc as bacc
import concourse.tile as tile
import concourse.mybir as mybir
from concourse.bass_utils import run_bass_kernel_spmd
from concourse.hw_specs import get_activation_tables

dt = mybir.dt
f32 = dt.float32
Alu = mybir.AluOpType
Act = mybir.ActivationFunctionType
AX = mybir.AxisListType

# problem constants (hardcoded per task instructions)
B, D, C = 4096, 2048, 1000
NCORES = 8
BL = B // NCORES          # 512 rows per core
NSEG = 4                  # 4 partition-blocks of 128 rows
NHALF = 500               # matmul N tile (<=512 for fp32 moving operand)

ALPHA, TAU, KAPPA = 1.0, 6.0, 100
BETA, LR = 5.0, 0.1
B1, B2, EPS, WD = 0.9, 0.999, 1e-8, 0.01
WDF = 1.0 - LR * WD
KSQ = math.sqrt(1.0 - B2) / (1.0 - B1)   # Square scale: gsq=(1-B2)G^2 from gIn
POISON = -1.0e4
PI = math.pi

_CACHE = {}


class _Bacc(bacc.Bacc):
    """Bacc with activation-table preference: put the exp+ln set first so the
    table-load placement pass doesn't thrash between exp-only and ln-only
    sets (saves ~4 ACT_TABLE_LOADs x 1.5us per iteration)."""

    def insert_act_table_loads(self):
        has_activation = any(
            isinstance(i, mybir.InstActivation)
            for b in self.main_func.blocks
            for i in b.instructions
        )
        if not has_activation:
            return
        # act_func_set_id is positional, so keep list order/length; shrink the
        # advertised membership of all but the preferred sets so the placement
        # pass always picks natural_log_exp_and_others (and trig_and_small for
        # the one init-time Sin). Runtime tables are resolved by id from the
        # real act_info.json, so execution is unaffected.
        AF = mybir.ActivationFunctionType
        newt = []
        for name, funcs in get_activation_tables(self.m.arch).items():
            if name == "natural_log_exp_and_others":
                newt.append((name, set(funcs)))
            elif name == "trig_and_small":
                newt.append((name, {AF.Sin}))
            else:
                newt.append((name, set()))
        _bass_rust.insert_act_table_loads(self, newt)


def _iter_consts():
    t = np.arange(1, KAPPA + 1, dtype=np.float64)
    bc1 = 1.0 - B1 ** t
    bc2 = 1.0 - B2 ** t
    lr_t = LR / bc1
    sqv_bias = -0.5 * np.log(bc2)
    den_bias = EPS / lr_t
    return lr_t, sqv_bias, den_bias


def _build():
    nc = _Bacc("TRN2", target_bir_lowering=False, debug=False,
               num_devices=NCORES)

    xT_d = nc.dram_tensor("xT", [D, BL], f32, kind="ExternalInput")
    W_d = nc.dram_tensor("W", [D, C], f32, kind="ExternalInput")
    b_d = nc.dram_tensor("b", [1, C], f32, kind="ExternalInput")
    eye_d = nc.dram_tensor("eye", [128, 128], f32, kind="ExternalInput")
    cst_d = nc.dram_tensor("cst", [128, 2 * KAPPA + 4], f32, kind="ExternalInput")
    out_d = nc.dram_tensor("out", [BL, C], f32, kind="ExternalOutput")

    lr_t, _sqv_bias, _den_bias = _iter_consts()

    with tile.TileContext(nc) as tc:
        with (
            tc.tile_pool(name="pers", bufs=1) as pers,

            tc.tile_pool(name="dr", bufs=4, space="DRAM") as dr,
            tc.tile_pool(name="sm", bufs=4) as sm,
        ):
            # ---------------- persistent state ----------------
            u = pers.tile([128, NSEG * C], f32, tag="u")
            m = pers.tile([128, NSEG * C], f32, tag="m")
            v = pers.tile([128, NSEG * C], f32, tag="v")
            ohm = pers.tile([128, NSEG * C], f32, tag="ohm")
            e = pers.tile([128, NSEG * C], f32, tag="e")   # also init scratch

            u3 = u[:].rearrange("p (s c) -> p s c", s=NSEG)
            e3 = e[:].rearrange("p (s c) -> p s c", s=NSEG)
            ohm3 = ohm[:].rearrange("p (s c) -> p s c", s=NSEG)
            # half views for the pipelined AdamW tail
            u2 = u[:].rearrange("p (h c) -> p h c", h=2)
            m2 = m[:].rearrange("p (h c) -> p h c", h=2)
            v2 = v[:].rearrange("p (h c) -> p h c", h=2)
            e2 = e[:].rearrange("p (h c) -> p h c", h=2)

            # persistent smalls
            u_y = pers.tile([128, NSEG], f32, tag="u_y")
            m_y = pers.tile([128, NSEG], f32, tag="m_y")
            v_y = pers.tile([128, NSEG], f32, tag="v_y")
            l_trg = pers.tile([128, NSEG], f32, tag="l_trg")
            ptA = pers.tile([128, 32], f32, tag="ptA")
            cst = pers.tile([128, 2 * KAPPA + 4], f32, tag="cst")
            eye = pers.tile([128, 128], f32, tag="eye")
            ones_r = pers.tile([1, 128], f32, tag="ones_r")
            ones_c = pers.tile([128, 1], f32, tag="ones_c")

            nc.sync.dma_start(cst[:], cst_d[:])
            nc.sync.dma_start(eye[:], eye_d[:])
            nc.vector.memset(ones_r[:], 1.0)
            nc.vector.memset(ones_c[:], 1.0)
            big30 = pers.tile([128, NSEG], f32, tag="big30")
            nc.vector.memset(big30[:], 1.0e30)
            ones_sq = pers.tile([128, 128], f32, tag="ones_sq")
            nc.vector.memset(ones_sq[:], 1.0)
            b37 = cst[:, 0:1]      # 1e-37  (Ln bias)

            # ---------------- init: z = x@W + b ----------------
            with (
                tc.tile_pool(name="init", bufs=1) as ini,
                tc.tile_pool(name="wstream", bufs=3) as wst,
                tc.tile_pool(name="psz", bufs=1, space="PSUM") as psz,
            ):
                xT = ini.tile([128, 16 * BL], f32, tag="xT")
                xT3 = xT[:].rearrange("p (k r) -> p k r", k=16)
                nc.sync.dma_start(
                    xT3, xT_d[:].rearrange("(k p) r -> p k r", p=128)
                )
                b_sb = ini.tile([1, C], f32, tag="b_sb")
                nc.sync.dma_start(b_sb[:], b_d[:])

                for nb in range(2):
                    zps = [
                        psz.tile([128, NHALF], f32, tag=f"z{mb}",
                                name=f"zps{mb}_{nb}")
                        for mb in range(NSEG)
                    ]
                    for k in range(16):
                        wc = wst.tile([128, NHALF], f32, tag="wc")
                        nc.sync.dma_start(
                            wc[:],
                            W_d[:].rearrange("(k p) c -> p k c", p=128)
                            [:, k, nb * NHALF:(nb + 1) * NHALF],
                        )
                        for mb in range(NSEG):
                            nc.tensor.matmul(
                                zps[mb][:],
                                xT3[:, k, mb * 128:(mb + 1) * 128],
                                wc[:],
                                start=(k == 0), stop=False,
                            )
                    for mb in range(NSEG):
                        nc.tensor.matmul(
                            zps[mb][:], ones_r[:],
                            b_sb[:, nb * NHALF:(nb + 1) * NHALF],
                            start=False, stop=True,
                        )
                        nc.scalar.copy(
                            u3[:, mb, nb * NHALF:(nb + 1) * NHALF],
                            zps[mb][:],
                        )

            # ---------------- init: stats from z ----------------
            rmz = sm.tile([128, NSEG], f32, tag="rmz")
            nc.vector.tensor_reduce(rmz[:], u3, axis=AX.X, op=Alu.max)
            sz = sm.tile([128, NSEG], f32, tag="sz")
            for s in range(NSEG):
                nc.scalar.activation(e3[:, s], u3[:, s], Act.Exp,
                                     accum_out=sz[:, s:s + 1])
            nc.vector.tensor_copy(u_y[:], rmz[:])
            for s in range(NSEG):
                nc.vector.tensor_scalar(ohm3[:, s], u3[:, s], rmz[:, s:s + 1],
                                        None, Alu.is_equal)
            # poison y column:  u += ohm * POISON
            nc.vector.scalar_tensor_tensor(u[:], ohm[:], POISON, u[:],
                                           Alu.mult, Alu.add)
            rsz = sm.tile([128, NSEG], f32, tag="rsz")
            nc.vector.reciprocal(rsz[:], sz[:])
            erm = sm.tile([128, NSEG], f32, tag="erm")
            nc.scalar.activation(erm[:], rmz[:], Act.Exp)
            p_trg = sm.tile([128, NSEG], f32, tag="p_trg")
            nc.vector.tensor_tensor(p_trg[:], erm[:], rsz[:], Alu.mult)
            lnsz = sm.tile([128, NSEG], f32, tag="lnsz")
            nc.scalar.activation(lnsz[:], sz[:], Act.Ln)
            l_org = sm.tile([128, NSEG], f32, tag="l_org")
            nc.vector.tensor_tensor(l_org[:], lnsz[:], rmz[:], Alu.subtract)
            fl = sm.tile([128, NSEG], f32, tag="fl")
            fk = sm.tile([128, NSEG], f32, tag="fk")
            nc.vector.tensor_scalar(fl[:], l_org[:], TAU, None, Alu.is_ge)
            for k in (2.0, 3.0):
                nc.vector.tensor_scalar(fk[:], l_org[:], k * TAU, None, Alu.is_ge)
                nc.vector.tensor_tensor(fl[:], fl[:], fk[:], Alu.add)
            l_atr = sm.tile([128, NSEG], f32, tag="l_atr")
            nc.vector.tensor_scalar(l_atr[:], fl[:], 0.5, TAU, Alu.add, Alu.mult)
            dla = sm.tile([128, NSEG], f32, tag="dla")
            nc.vector.tensor_tensor(dla[:], l_org[:], l_atr[:], Alu.subtract)
            th = sm.tile([128, NSEG], f32, tag="th")
            nc.vector.tensor_scalar(th[:], dla[:], -2.0 * PI / TAU, PI,
                                    Alu.mult, Alu.add)
            sth = sm.tile([128, NSEG], f32, tag="sth")
            nc.scalar.activation(sth[:], th[:], Act.Sin)
            sths = sm.tile([128, NSEG], f32, tag="sths")
            nc.vector.tensor_scalar(sths[:], sth[:], -ALPHA * TAU, None, Alu.mult)
            nc.vector.tensor_tensor(l_trg[:], l_org[:], sths[:], Alu.add)

            nc.vector.memset(m[:], 0.0)
            nc.vector.memset(v[:], 0.0)
            nc.vector.memset(m_y[:], 0.0)
            nc.vector.memset(v_y[:], 0.0)

            # allgather p_trg (constant over iterations)
            pt_in = dr.tile([BL], f32, tag="pt_in")
            pt_out = dr.tile([B], f32, tag="pt_out")
            nc.sync.dma_start(
                pt_in[:].rearrange("(p s) -> p s", p=128), p_trg[:]
            )
            nc.gpsimd.collective_compute(
                "AllGather", Alu.bypass,
                replica_groups=[list(range(NCORES))],
                ins=[pt_in[:].opt()], outs=[pt_out[:].opt()],
            )
            nc.sync.dma_start(
                ptA[:], pt_out[:].rearrange("(p c) -> p c", p=128)
            )

            # iteration-scratch bigs
            with (
                tc.tile_pool(name="big2", bufs=1) as big2,
                tc.tile_pool(name="ps", bufs=2, space="PSUM") as ps,
            ):
                msk = big2.tile([128, NSEG * C], f32, tag="msk")
                gIn = big2.tile([128, NSEG * C], f32, tag="gIn")
                msk3 = msk[:].rearrange("p (s c) -> p s c", s=NSEG)
                g3 = gIn[:].rearrange("p (s c) -> p s c", s=NSEG)
                msk2 = msk[:].rearrange("p (h c) -> p h c", h=2)
                g2 = gIn[:].rearrange("p (h c) -> p h c", h=2)

                # ---------------- 100 iterations ----------------
                for tt in range(KAPPA):
                    ilr = 1.0 / lr_t[tt]
                    sqb = cst[:, 4 + tt:5 + tt]            # -0.5*ln(bc2_t)
                    dnb = cst[:, 4 + KAPPA + tt:5 + KAPPA + tt]   # EPS/lr_t

                    # phase A: exp + row stats
                    s_m = sm.tile([128, NSEG], f32, tag="s_m")
                    for s in range(NSEG):
                        nc.scalar.activation(e3[:, s], u3[:, s], Act.Exp,
                                             accum_out=s_m[:, s:s + 1])
                    rmu = sm.tile([128, NSEG], f32, tag="rmu")
                    for s in range(NSEG):
                        nc.vector.tensor_reduce(rmu[:, s:s + 1], u3[:, s],
                                                axis=AX.X, op=Alu.max)

                    e_y = sm.tile([128, NSEG], f32, tag="e_y")
                    nc.scalar.activation(e_y[:], u_y[:], Act.Exp)
                    s_f = sm.tile([128, NSEG], f32, tag="s_f")
                    nc.vector.tensor_tensor(s_f[:], s_m[:], e_y[:], Alu.add)
                    rs = sm.tile([128, NSEG], f32, tag="rs")
                    nc.vector.reciprocal(rs[:], s_f[:])
                    ln_s = sm.tile([128, NSEG], f32, tag="ln_s")
                    nc.scalar.activation(ln_s[:], s_f[:], Act.Ln)
                    ce = sm.tile([128, NSEG], f32, tag="ce")
                    nc.vector.tensor_tensor(ce[:], ln_s[:], u_y[:], Alu.subtract)
                    dce = sm.tile([128, NSEG], f32, tag="dce")
                    nc.vector.tensor_tensor(dce[:], ce[:], l_trg[:], Alu.subtract)
                    sgp = sm.tile([128, NSEG], f32, tag="sgp")
                    nc.vector.tensor_scalar(sgp[:], dce[:], 0.0, None, Alu.is_gt)
                    sgn = sm.tile([128, NSEG], f32, tag="sgn")
                    nc.vector.tensor_scalar(sgn[:], dce[:], 0.0, None, Alu.is_lt)
                    sg1 = sm.tile([128, NSEG], f32, tag="sg1")
                    nc.vector.tensor_tensor(sg1[:], sgp[:], sgn[:], Alu.subtract)
                    wm = sm.tile([128, NSEG], f32, tag="wm")
                    nc.vector.tensor_tensor(wm[:], rmu[:], u_y[:], Alu.is_gt)
                    rme = sm.tile([128, NSEG], f32, tag="rme")
                    nc.scalar.activation(rme[:], rmu[:], Act.Exp)
                    rmef = sm.tile([128, NSEG], f32, tag="rmef")
                    nc.vector.tensor_tensor(rmef[:], rme[:], e_y[:], Alu.max)
                    rpm = sm.tile([128, NSEG], f32, tag="rpm")
                    nc.vector.tensor_tensor(rpm[:], rmef[:], rs[:], Alu.mult)
                    wmb = sm.tile([128, NSEG], f32, tag="wmb")
                    nc.vector.tensor_scalar(wmb[:], wm[:], -1.0e30, 1.0e30,
                                            Alu.mult, Alu.add)
                    rmu_m = sm.tile([128, NSEG], f32, tag="rmu_m")
                    nc.vector.tensor_tensor(rmu_m[:], rmu[:], wmb[:], Alu.add)

                    # local max over 512 rows -> 4 values -> allgather
                    ptr = ps.tile([4, 128], f32, tag="ptr")
                    nc.tensor.transpose(ptr[:], rpm[:], eye[:])
                    lm4 = sm.tile([4, 1], f32, tag="lm4")
                    nc.vector.tensor_reduce(lm4[:], ptr[:], axis=AX.X, op=Alu.max)
                    cc_in = dr.tile([4], f32, tag="cc_in")
                    cc_out = dr.tile([32], f32, tag="cc_out")
                    nc.sync.dma_start(cc_in[:].rearrange("(p s) -> p s", p=4),
                                      lm4[:])
                    nc.gpsimd.collective_compute(
                        "AllGather", Alu.bypass,
                        replica_groups=[list(range(NCORES))],
                        ins=[cc_in[:].opt()], outs=[cc_out[:].opt()],
                    )
                    gm = sm.tile([1, 32], f32, tag="gm")
                    nc.sync.dma_start(gm[:],
                                      cc_out[:].rearrange("(a b) -> a b", a=1))

                    # global scalar chain
                    spm = sm.tile([1, 1], f32, tag="spm")
                    nc.vector.tensor_reduce(spm[:], gm[:], axis=AX.X, op=Alu.max)
                    spmb = ps.tile([128, 1], f32, tag="spmb")
                    nc.tensor.matmul(spmb[:], ones_r[:], spm[:],
                                     start=True, stop=True)
                    ind = sm.tile([128, NSEG], f32, tag="ind")
                    nc.vector.tensor_scalar(ind[:], rpm[:], spmb[:, 0:1], None,
                                            Alu.is_equal)
                    sca = sm.tile([128, 32], f32, tag="sca")
                    s2a = sm.tile([128, 1], f32, tag="s2a")
                    nc.vector.tensor_scalar(sca[:], ptA[:], spmb[:, 0:1], None,
                                            Alu.is_lt, Alu.add, accum_out=s2a[:])
                    scb = sm.tile([128, 32], f32, tag="scb")
                    s2b = sm.tile([128, 1], f32, tag="s2b")
                    nc.vector.tensor_scalar(scb[:], ptA[:], spmb[:, 0:1], None,
                                            Alu.is_gt, Alu.add, accum_out=s2b[:])
                    s2d = sm.tile([128, 1], f32, tag="s2d")
                    nc.vector.tensor_tensor(s2d[:], s2a[:], s2b[:], Alu.subtract)
                    s2bc = ps.tile([128, 1], f32, tag="s2bc")
                    nc.tensor.matmul(s2bc[:], ones_sq[:], s2d[:],
                                     start=True, stop=True)
                    b1t = sm.tile([128, NSEG], f32, tag="b1t")
                    nc.vector.tensor_scalar(b1t[:], ind[:], spmb[:, 0:1], None,
                                            Alu.mult)
                    b1v = sm.tile([128, NSEG], f32, tag="b1v")
                    nc.vector.tensor_scalar(b1v[:], b1t[:], s2bc[:, 0:1],
                                            BETA * (1.0 - B1),
                                            Alu.mult, Alu.mult)
                    # rw = rs * ((1-B1)*sign1 - b1v)
                    wv = sm.tile([128, NSEG], f32, tag="wv")
                    nc.vector.scalar_tensor_tensor(wv[:], sg1[:], 1.0 - B1,
                                                   b1v[:], Alu.mult, Alu.subtract)
                    rw = sm.tile([128, NSEG], f32, tag="rw")
                    nc.vector.tensor_tensor(rw[:], wv[:], rs[:], Alu.mult)

                    # gIn = e (x) rw + mask (x) b1v   (mask = is_eq(u, rmu'))
                    for s in range(NSEG):
                        nc.vector.tensor_scalar(
                            msk3[:, s], u3[:, s], rmu_m[:, s:s + 1],
                            b1v[:, s:s + 1], Alu.is_equal, Alu.mult)
                    for s in range(NSEG):
                        nc.vector.scalar_tensor_tensor(
                            g3[:, s], e3[:, s], rw[:, s:s + 1], msk3[:, s],
                            Alu.mult, Alu.add)

                    # AdamW tail, two pipelined halves
                    gsq2 = msk2   # reuse msk
                    lnv2 = e2     # reuse e
                    sqv2 = g2     # reuse gIn (dead after m' & gsq)
                    den2 = e2     # reuse e  (lnv dead after sqv)
                    rdn2 = g2     # reuse gIn (sqv dead after den)
                    t32 = msk2    # reuse msk (gsq dead after v')
                    for h in range(2):
                        nc.vector.scalar_tensor_tensor(
                            m2[:, h], m2[:, h], B1, g2[:, h],
                            Alu.mult, Alu.add)
                        nc.scalar.activation(gsq2[:, h], g2[:, h], Act.Square,
                                             scale=KSQ)
                        nc.vector.scalar_tensor_tensor(
                            v2[:, h], v2[:, h], B2, gsq2[:, h],
                            Alu.mult, Alu.add)
                        nc.scalar.activation(lnv2[:, h], v2[:, h], Act.Ln,
                                             bias=b37)
                        nc.scalar.activation(sqv2[:, h], lnv2[:, h], Act.Exp,
                                             scale=0.5, bias=sqb)
                        nc.vector.tensor_scalar(den2[:, h], sqv2[:, h], EPS,
                                                ilr, Alu.add, Alu.mult)
                        nc.vector.reciprocal_approx_fast(rdn2[:, h], den2[:, h])
                        nc.vector.tensor_tensor(t32[:, h], m2[:, h], rdn2[:, h],
                                                Alu.mult)
                        for s in (2 * h, 2 * h + 1):
                            nc.vector.scalar_tensor_tensor(
                                u3[:, s], u3[:, s], WDF, msk3[:, s],
                                Alu.mult, Alu.subtract)

                    # side state (exact y-column trajectory)
                    p_y = sm.tile([128, NSEG], f32, tag="p_y")
                    nc.vector.tensor_tensor(p_y[:], e_y[:], rs[:], Alu.mult)
                    ty1 = sm.tile([128, NSEG], f32, tag="ty1")
                    nc.vector.tensor_scalar(ty1[:], p_y[:], 1.0, None,
                                            Alu.subtract)
                    ty2 = sm.tile([128, NSEG], f32, tag="ty2")
                    nc.vector.tensor_tensor(ty2[:], ty1[:], sg1[:], Alu.mult)
                    ty3 = sm.tile([128, NSEG], f32, tag="ty3")
                    nc.vector.tensor_scalar(ty3[:], wm[:], -1.0, 1.0,
                                            Alu.mult, Alu.add)
                    ty4 = sm.tile([128, NSEG], f32, tag="ty4")
                    nc.vector.tensor_tensor(ty4[:], ty3[:], p_y[:], Alu.subtract)
                    ty5 = sm.tile([128, NSEG], f32, tag="ty5")
                    nc.vector.tensor_tensor(ty5[:], ty4[:], b1v[:], Alu.mult)
                    gyi = sm.tile([128, NSEG], f32, tag="gyi")
                    nc.vector.scalar_tensor_tensor(gyi[:], ty2[:], 1.0 - B1,
                                                   ty5[:], Alu.mult, Alu.add)
                    nc.vector.scalar_tensor_tensor(m_y[:], m_y[:], B1, gyi[:],
                                                   Alu.mult, Alu.add)
                    gys = sm.tile([128, NSEG], f32, tag="gys")
                    nc.scalar.activation(gys[:], gyi[:], Act.Square, scale=KSQ)
                    nc.vector.scalar_tensor_tensor(v_y[:], v_y[:], B2, gys[:],
                                                   Alu.mult, Alu.add)
                    lny = sm.tile([128, NSEG], f32, tag="lny")
                    nc.scalar.activation(lny[:], v_y[:], Act.Ln, bias=b37)
                    sqy = sm.tile([128, NSEG], f32, tag="sqy")
                    nc.scalar.activation(sqy[:], lny[:], Act.Exp, scale=0.5,
                                         bias=sqb)
                    dny = sm.tile([128, NSEG], f32, tag="dny")
                    nc.vector.tensor_scalar(dny[:], sqy[:], EPS, ilr,
                                            Alu.add, Alu.mult)
                    rdy = sm.tile([128, NSEG], f32, tag="rdy")
                    nc.vector.reciprocal_approx_fast(rdy[:], dny[:])
                    t3y = sm.tile([128, NSEG], f32, tag="t3y")
                    nc.vector.tensor_tensor(t3y[:], m_y[:], rdy[:], Alu.mult)
                    nc.vector.scalar_tensor_tensor(u_y[:], u_y[:], WDF, t3y[:],
                                                   Alu.mult, Alu.subtract)

                # ---------------- merge y column back, write out ----------
                ohm2 = e
                nc.vector.tensor_scalar(ohm2[:], ohm[:], -1.0, 1.0,
                                        Alu.mult, Alu.add)
                for s in range(NSEG):
                    nc.vector.tensor_scalar(msk3[:, s], ohm3[:, s],
                                            u_y[:, s:s + 1], None, Alu.mult)
                ub = gIn
                nc.vector.tensor_tensor(ub[:], u[:], ohm2[:], Alu.mult)
                nc.vector.tensor_tensor(ub[:], ub[:], msk[:], Alu.add)
                nc.sync.dma_start(
                    out_d[:].rearrange("(s p) c -> p s c", p=128),
                    ub[:].rearrange("p (s c) -> p s c", s=NSEG),
                )

    nc.compile()
    return nc


def _get_nc():
    if "nc" not in _CACHE:
        _CACHE["nc"] = _build()
    return _CACHE["nc"]


def _make_in_maps(x, W, b):
    x = np.ascontiguousarray(np.asarray(x, np.float32))
    W = np.ascontiguousarray(np.asarray(W, np.float32))
    b = np.ascontiguousarray(np.asarray(b, np.float32)).reshape(1, C)
    eye = np.eye(128, dtype=np.float32)

    _lr_t, sqv_bias, den_bias = _iter_consts()
    cst = np.zeros((128, 2 * KAPPA + 4), np.float32)
    cst[:, 0] = 1e-37
    cst[:, 4:4 + KAPPA] = np.float32(sqv_bias)[None, :]
    cst[:, 4 + KAPPA:4 + 2 * KAPPA] = np.float32(den_bias)[None, :]

    in_maps = []
    for c in range(NCORES):
        xs = x[c * BL:(c + 1) * BL]
        in_maps.append({
            "xT": np.ascontiguousarray(xs.T),
            "W": W, "b": b, "eye": eye, "cst": cst,
        })
    return in_maps


def _run(x, W, b, trace=False):
    nc = _get_nc()
    in_maps = _make_in_maps(x, W, b)
    res = run_bass_kernel_spmd(nc, in_maps, core_ids=list(range(NCORES)),
                               trace=trace)
    out = np.concatenate([res.results[c]["out"] for c in range(NCORES)], axis=0)
    return out, res


def kernel(**inputs):
    out, _ = _run(inputs["x"], inputs["W"], inputs["b"])
    return out



# revision 5
# speedup vs baseline: 1.0723x; 1.0723x over previous
"""AAA-protected classifier kernel for Trainium2, 8 NeuronCores.

Math (see reference): z = x@W+b; y = argmax(z); 100 AdamW steps on u (init z)
minimizing  sum_i |ce(u_i,y_i) - l_trg_i| + BETA*|max_all(softmax(u)) - p_trg_i|.

Gradient (faithful to jax autodiff), with e = exp(u), s = rowsum e, p = e/s:
  G = sign1_i * (p - onehot_y) + C1 * ind_i * (onehot_jmax - p)
  sign1_i = sign(ce_i - l_trg_i);  s_pmax = global max p (one collective/iter)
  S2 = sum_i sign(s_pmax - p_trg_i)  (from the once-gathered p_trg vector)
  C1 = BETA * S2 * s_pmax;  ind_i = row i attains the global max.

This version reformulates AdamW so every bulk pass is a 2x/4x-capable DVE op
(TT/TS only, no scalar_tensor_tensor on big tiles), with per-iteration scalar
algebra folded into activation scale/bias immediates:
  - deferred weight decay: store utl = u / WDF^t  (u-update = plain TT subtract)
  - scaled momentum (bf16): mtl = m / B1^t        (m-update = plain TT add)
  - 1/(sqrt(vhat)+EPS) ~= exp(-0.5*ln(v) + C_t)   (EPS dropped; Act engine)
  - gsq on Act engine: (sqrt(c_t)*gIn')^2; v-update on Pool (gpsimd) engine
Per-row weights fold B1^{-t} so gIn' = gIn * B1^{-t} flows into mtl directly.

The y-column of each row is poisoned to -1e4 in the main tensor (exp -> 0,
never wins a max, gradient stays exactly 0) and tracked exactly in [128,4]
side state (u_y, m_y, v_y) in TRUE (unscaled) space, merged back at the end.

Sharding: batch 4096 -> 512 rows/core (data parallel), W/b replicated.
Layout per core: [128 partitions, 4 segments, 1000 classes]; row = s*128+p.
"""

import math
import numpy as np

import bass_rust as _bass_rust
import concourse.bass as bass
import concourse.bacc as bacc
import concourse.tile as tile
import concourse.mybir as mybir
from concourse.bass_utils import run_bass_kernel_spmd
from concourse.hw_specs import get_activation_tables

dt = mybir.dt
f32 = dt.float32
bf16 = dt.bfloat16
Alu = mybir.AluOpType
Act = mybir.ActivationFunctionType
AX = mybir.AxisListType

# problem constants (hardcoded per task instructions)
B, D, C = 4096, 2048, 1000
NCORES = 8
BL = B // NCORES          # 512 rows per core
NSEG = 4                  # 4 partition-blocks of 128 rows
NHALF = 500               # matmul N tile (<=512 for fp32 moving operand)

ALPHA, TAU, KAPPA = 1.0, 6.0, 100
BETA, LR = 5.0, 0.1
B1, B2, EPS, WD = 0.9, 0.999, 1e-8, 0.01
WDF = 1.0 - LR * WD
POISON = -1.0e4
PI = math.pi

_CACHE = {}


class _Bacc(bacc.Bacc):
    """Bacc with activation-table preference: pin the exp+ln set so the
    table-load placement pass never thrashes between sets."""

    def insert_act_table_loads(self):
        has_activation = any(
            isinstance(i, mybir.InstActivation)
            for b in self.main_func.blocks
            for i in b.instructions
        )
        if not has_activation:
            return
        AF = mybir.ActivationFunctionType
        newt = []
        for name, funcs in get_activation_tables(self.m.arch).items():
            if name == "natural_log_exp_and_others":
                newt.append((name, set(funcs)))
            elif name == "trig_and_small":
                newt.append((name, {AF.Sin}))
            else:
                newt.append((name, set()))
        _bass_rust.insert_act_table_loads(self, newt)


def _iter_consts():
    """Per-iteration folded constants (float64 precomputation).

    Iteration tt (0-based), producing step t = tt+1:
      u_true(tt) = WDF^tt * utl          (state entering the iteration)
      e = exp(u_true) -> Act scale SC_E[tt] = WDF^tt
      gIn' = gIn_true * B1^{-t}: folded into row weights (SC1/SCB below)
      mtl += gIn'                        (mtl = m_true * B1^{-t})
      gsq = (SQC[tt]*gIn')^2 = (1-B2)*G^2,  SQC = sqrt(1-B2)*B1^t/(1-B1)
      v = B2*v + gsq                     (v = (1-B2)-scaled, unscaled in t)
      rdn = exp(-0.5*ln(v+1e-37) + LNC[tt])
          = lr_t * B1^t * sqrt((1-B2)*bc2)/((1-B1)*bc1) / sqrt(v) / WDF^t
      utl -= mtl * rdn
    Side (true-space) y-state keeps EPS and the baseline ln/exp sqrt:
      sqb = -0.5*ln(bc2), den = (sqv+EPS)/lr_t
    """
    t = np.arange(1, KAPPA + 1, dtype=np.float64)
    bc1 = 1.0 - B1 ** t
    bc2 = 1.0 - B2 ** t
    lr_t = LR / bc1
    sc_e = WDF ** (t - 1.0)             # WDF^tt
    sc_g = B1 ** (-t)                   # B1^{-t}
    # v is stored B2-descaled (vbar = v/B2^t) so its update is a plain add:
    #   vbar += (SQC*gi)^2,  SQC = sqrt((1-B2)*B2^{-t})*B1^t/(1-B1)
    sqc = np.sqrt((1.0 - B2) * B2 ** (-t)) * (B1 ** t) / (1.0 - B1)
    # NOTE: reference's m/v already carry (1-B1)/(1-B2); mhat = m/bc1.
    lnc = np.log(LR) + t * np.log(B1) + 0.5 * np.log(bc2) \
        - 0.5 * t * np.log(B2) - np.log(bc1) - t * np.log(WDF)
    # y-side (baseline exact form)
    sqv_bias = -0.5 * np.log(bc2)
    return dict(lr_t=lr_t, sc_e=sc_e, sc_g=sc_g, sqc=sqc, lnc=lnc,
                sqv_bias=sqv_bias)


def _build():
    nc = _Bacc("TRN2", target_bir_lowering=False, debug=False,
               num_devices=NCORES)

    xT_d = nc.dram_tensor("xT", [D, BL], f32, kind="ExternalInput")
    W_d = nc.dram_tensor("W", [D, C], f32, kind="ExternalInput")
    b_d = nc.dram_tensor("b", [1, C], f32, kind="ExternalInput")
    eye_d = nc.dram_tensor("eye", [128, 128], f32, kind="ExternalInput")
    cst_d = nc.dram_tensor("cst", [128, 3 * KAPPA + 4], f32, kind="ExternalInput")
    out_d = nc.dram_tensor("out", [BL, C], f32, kind="ExternalOutput")

    cc = _iter_consts()
    lr_t = cc["lr_t"]
    sc_e = cc["sc_e"]
    sc_g = cc["sc_g"]
    sqc = cc["sqc"]

    with tile.TileContext(nc) as tc:
        with (
            tc.tile_pool(name="pers", bufs=1) as pers,
            tc.tile_pool(name="dr", bufs=4, space="DRAM") as dr,
            tc.tile_pool(name="sm", bufs=4) as sm,
        ):
            # ---------------- persistent state ----------------
            u = pers.tile([128, NSEG * C], f32, tag="u")       # utl (scaled)
            v = pers.tile([128, NSEG * C], f32, tag="v")
            ohm = pers.tile([128, NSEG * C], f32, tag="ohm")
            gsq = pers.tile([128, NSEG * C], f32, tag="gsq")   # also lnv
            mt = pers.tile([128, NSEG * C], bf16, tag="mt")    # mtl (scaled)
            e = pers.tile([128, NSEG * C], bf16, tag="e")
            ew = pers.tile([128, NSEG * C], bf16, tag="ew")    # also t3
            m0b = pers.tile([128, NSEG * C], bf16, tag="m0b")  # also rdn
            gi = pers.tile([128, NSEG * C], bf16, tag="gi")    # gIn'

            u3 = u[:].rearrange("p (s c) -> p s c", s=NSEG)
            e3 = e[:].rearrange("p (s c) -> p s c", s=NSEG)
            ohm3 = ohm[:].rearrange("p (s c) -> p s c", s=NSEG)
            m03 = m0b[:].rearrange("p (s c) -> p s c", s=NSEG)
            ew3 = ew[:].rearrange("p (s c) -> p s c", s=NSEG)
            u2 = u[:].rearrange("p (h c) -> p h c", h=2)
            v2 = v[:].rearrange("p (h c) -> p h c", h=2)
            gsq2 = gsq[:].rearrange("p (h c) -> p h c", h=2)
            mt2 = mt[:].rearrange("p (h c) -> p h c", h=2)
            ew2 = ew[:].rearrange("p (h c) -> p h c", h=2)
            m0b2 = m0b[:].rearrange("p (h c) -> p h c", h=2)
            gi2 = gi[:].rearrange("p (h c) -> p h c", h=2)

            # persistent smalls (true-space y side state)
            u_y = pers.tile([128, NSEG], f32, tag="u_y")
            m_y = pers.tile([128, NSEG], f32, tag="m_y")
            v_y = pers.tile([128, NSEG], f32, tag="v_y")
            l_trg = pers.tile([128, NSEG], f32, tag="l_trg")
            e_y = pers.tile([128, NSEG], f32, tag="e_y")
            ptA = pers.tile([128, 32], f32, tag="ptA")
            cst = pers.tile([128, 3 * KAPPA + 4], f32, tag="cst")
            eye = pers.tile([128, 128], f32, tag="eye")
            ones_r = pers.tile([1, 128], f32, tag="ones_r")
            ones_sq = pers.tile([128, 128], f32, tag="ones_sq")

            nc.sync.dma_start(cst[:], cst_d[:])
            nc.sync.dma_start(eye[:], eye_d[:])
            nc.vector.memset(ones_r[:], 1.0)
            nc.vector.memset(ones_sq[:], 1.0)
            b37 = cst[:, 0:1]      # 1e-37  (Ln bias)

            # ---------------- init: z = x@W + b ----------------
            with (
                tc.tile_pool(name="init", bufs=1) as ini,
                tc.tile_pool(name="wstream", bufs=3) as wst,
                tc.tile_pool(name="psz", bufs=1, space="PSUM") as psz,
            ):
                xT = ini.tile([128, 16 * BL], f32, tag="xT")
                xT3 = xT[:].rearrange("p (k r) -> p k r", k=16)
                nc.sync.dma_start(
                    xT3, xT_d[:].rearrange("(k p) r -> p k r", p=128)
                )
                b_sb = ini.tile([1, C], f32, tag="b_sb")
                nc.sync.dma_start(b_sb[:], b_d[:])

                for nb in range(2):
                    zps = [
                        psz.tile([128, NHALF], f32, tag=f"z{mb}",
                                name=f"zps{mb}_{nb}")
                        for mb in range(NSEG)
                    ]
                    for k in range(16):
                        wc = wst.tile([128, NHALF], f32, tag="wc")
                        nc.sync.dma_start(
                            wc[:],
                            W_d[:].rearrange("(k p) c -> p k c", p=128)
                            [:, k, nb * NHALF:(nb + 1) * NHALF],
                        )
                        for mb in range(NSEG):
                            nc.tensor.matmul(
                                zps[mb][:],
                                xT3[:, k, mb * 128:(mb + 1) * 128],
                                wc[:],
                                start=(k == 0), stop=False,
                            )
                    for mb in range(NSEG):
                        nc.tensor.matmul(
                            zps[mb][:], ones_r[:],
                            b_sb[:, nb * NHALF:(nb + 1) * NHALF],
                            start=False, stop=True,
                        )
                        nc.scalar.copy(
                            u3[:, mb, nb * NHALF:(nb + 1) * NHALF],
                            zps[mb][:],
                        )

            # ---------------- init: stats from z ----------------
            # (use gsq as fp32 scratch for exp(z))
            ez3 = gsq[:].rearrange("p (s c) -> p s c", s=NSEG)
            rmz = sm.tile([128, NSEG], f32, tag="rmz")
            nc.vector.tensor_reduce(rmz[:], u3, axis=AX.X, op=Alu.max)
            sz = sm.tile([128, NSEG], f32, tag="sz")
            for s in range(NSEG):
                nc.scalar.activation(ez3[:, s], u3[:, s], Act.Exp,
                                     accum_out=sz[:, s:s + 1])
            nc.vector.tensor_copy(u_y[:], rmz[:])
            for s in range(NSEG):
                nc.vector.tensor_scalar(ohm3[:, s], u3[:, s], rmz[:, s:s + 1],
                                        None, Alu.is_equal)
            # poison y column:  u += ohm * POISON   (t=0: utl == u_true)
            nc.vector.scalar_tensor_tensor(u[:], ohm[:], POISON, u[:],
                                           Alu.mult, Alu.add)
            rsz = sm.tile([128, NSEG], f32, tag="rsz")
            nc.vector.reciprocal(rsz[:], sz[:])
            erm = sm.tile([128, NSEG], f32, tag="erm")
            nc.scalar.activation(erm[:], rmz[:], Act.Exp)
            p_trg = sm.tile([128, NSEG], f32, tag="p_trg")
            nc.vector.tensor_tensor(p_trg[:], erm[:], rsz[:], Alu.mult)
            lnsz = sm.tile([128, NSEG], f32, tag="lnsz")
            nc.scalar.activation(lnsz[:], sz[:], Act.Ln)
            l_org = sm.tile([128, NSEG], f32, tag="l_org")
            nc.vector.tensor_tensor(l_org[:], lnsz[:], rmz[:], Alu.subtract)
            fl = sm.tile([128, NSEG], f32, tag="fl")
            fk = sm.tile([128, NSEG], f32, tag="fk")
            nc.vector.tensor_scalar(fl[:], l_org[:], TAU, None, Alu.is_ge)
            for k in (2.0, 3.0):
                nc.vector.tensor_scalar(fk[:], l_org[:], k * TAU, None, Alu.is_ge)
                nc.vector.tensor_tensor(fl[:], fl[:], fk[:], Alu.add)
            l_atr = sm.tile([128, NSEG], f32, tag="l_atr")
            nc.vector.tensor_scalar(l_atr[:], fl[:], 0.5, TAU, Alu.add, Alu.mult)
            dla = sm.tile([128, NSEG], f32, tag="dla")
            nc.vector.tensor_tensor(dla[:], l_org[:], l_atr[:], Alu.subtract)
            th = sm.tile([128, NSEG], f32, tag="th")
            nc.vector.tensor_scalar(th[:], dla[:], -2.0 * PI / TAU, PI,
                                    Alu.mult, Alu.add)
            sth = sm.tile([128, NSEG], f32, tag="sth")
            nc.scalar.activation(sth[:], th[:], Act.Sin)
            sths = sm.tile([128, NSEG], f32, tag="sths")
            nc.vector.tensor_scalar(sths[:], sth[:], -ALPHA * TAU, None, Alu.mult)
            nc.vector.tensor_tensor(l_trg[:], l_org[:], sths[:], Alu.add)

            nc.vector.memset(mt[:], 0.0)
            nc.vector.memset(v[:], 0.0)
            nc.vector.memset(m_y[:], 0.0)
            nc.vector.memset(v_y[:], 0.0)
            nc.scalar.activation(e_y[:], u_y[:], Act.Exp)

            # allgather p_trg (constant over iterations)
            pt_in = dr.tile([BL], f32, tag="pt_in")
            pt_out = dr.tile([B], f32, tag="pt_out")
            nc.sync.dma_start(
                pt_in[:].rearrange("(p s) -> p s", p=128), p_trg[:]
            )
            nc.gpsimd.collective_compute(
                "AllGather", Alu.bypass,
                replica_groups=[list(range(NCORES))],
                ins=[pt_in[:].opt()], outs=[pt_out[:].opt()],
            )
            nc.sync.dma_start(
                ptA[:], pt_out[:].rearrange("(p c) -> p c", p=128)
            )

            with tc.tile_pool(name="ps", bufs=2, space="PSUM") as ps:
                # ---------------- 100 iterations ----------------
                for tt in range(KAPPA):
                    SCE = float(sc_e[tt])        # exp scale  WDF^tt
                    SCG = float(sc_g[tt])        # B1^{-t}
                    SQC = float(sqc[tt])         # sqrt(1-B2)*B1^t/(1-B1)
                    lncb = cst[:, 4 + 2 * KAPPA + tt:5 + 2 * KAPPA + tt]
                    sqb = cst[:, 4 + tt:5 + tt]               # y-side
                    dnb = cst[:, 4 + KAPPA + tt:5 + KAPPA + tt]
                    ilr = 1.0 / lr_t[tt]

                    # phase A: exp + row stats  (e in bf16, true-space values)
                    s_m = sm.tile([128, NSEG], f32, tag="s_m")
                    for s in range(NSEG):
                        nc.scalar.activation(e3[:, s], u3[:, s], Act.Exp,
                                             scale=SCE,
                                             accum_out=s_m[:, s:s + 1])
                    rmu = sm.tile([128, NSEG], f32, tag="rmu")
                    for s in range(NSEG):
                        nc.vector.tensor_reduce(rmu[:, s:s + 1], u3[:, s],
                                                axis=AX.X, op=Alu.max)

                    # true-space row stats. e_y precomputed at end of prev iter.
                    s_f = sm.tile([128, NSEG], f32, tag="s_f")
                    nc.vector.tensor_tensor(s_f[:], s_m[:], e_y[:], Alu.add)
                    rs = sm.tile([128, NSEG], f32, tag="rs")
                    nc.vector.reciprocal(rs[:], s_f[:])
                    ln_s = sm.tile([128, NSEG], f32, tag="ln_s")
                    nc.scalar.activation(ln_s[:], s_f[:], Act.Ln)
                    ce = sm.tile([128, NSEG], f32, tag="ce")
                    nc.vector.tensor_tensor(ce[:], ln_s[:], u_y[:], Alu.subtract)
                    dce = sm.tile([128, NSEG], f32, tag="dce")
                    nc.vector.tensor_tensor(dce[:], ce[:], l_trg[:], Alu.subtract)
                    sg1 = sm.tile([128, NSEG], f32, tag="sg1")
                    nc.scalar.activation(sg1[:], dce[:], Act.Sign)
                    # rme = exp(true rmu) = Exp(rmu_scaled * SCE)
                    rme = sm.tile([128, NSEG], f32, tag="rme")
                    nc.scalar.activation(rme[:], rmu[:], Act.Exp, scale=SCE)
                    wm = sm.tile([128, NSEG], f32, tag="wm")
                    nc.vector.tensor_tensor(wm[:], rme[:], e_y[:], Alu.is_gt)
                    rmef = sm.tile([128, NSEG], f32, tag="rmef")
                    nc.vector.tensor_tensor(rmef[:], rme[:], e_y[:], Alu.max)
                    rpm = sm.tile([128, NSEG], f32, tag="rpm")
                    nc.vector.tensor_tensor(rpm[:], rmef[:], rs[:], Alu.mult)
                    wmb = sm.tile([128, NSEG], f32, tag="wmb")
                    nc.vector.tensor_scalar(wmb[:], wm[:], -1.0e30, 1.0e30,
                                            Alu.mult, Alu.add)
                    rmu_m = sm.tile([128, NSEG], f32, tag="rmu_m")
                    nc.vector.tensor_tensor(rmu_m[:], rmu[:], wmb[:], Alu.add)

                    # local max over 512 rows -> 4 values -> allgather
                    ptr = ps.tile([4, 128], f32, tag="ptr")
                    nc.tensor.transpose(ptr[:], rpm[:], eye[:])
                    lm4 = sm.tile([4, 1], f32, tag="lm4")
                    nc.vector.tensor_reduce(lm4[:], ptr[:], axis=AX.X, op=Alu.max)
                    cc_in = dr.tile([4], f32, tag="cc_in")
                    cc_out = dr.tile([32], f32, tag="cc_out")
                    nc.sync.dma_start(cc_in[:].rearrange("(p s) -> p s", p=4),
                                      lm4[:])
                    nc.gpsimd.collective_compute(
                        "AllGather", Alu.bypass,
                        replica_groups=[list(range(NCORES))],
                        ins=[cc_in[:].opt()], outs=[cc_out[:].opt()],
                    )
                    gm = sm.tile([1, 32], f32, tag="gm")
                    nc.sync.dma_start(gm[:],
                                      cc_out[:].rearrange("(a b) -> a b", a=1))

                    # global scalar chain
                    spm = sm.tile([1, 1], f32, tag="spm")
                    nc.vector.tensor_reduce(spm[:], gm[:], axis=AX.X, op=Alu.max)
                    spmb = ps.tile([128, 1], f32, tag="spmb")
                    nc.tensor.matmul(spmb[:], ones_r[:], spm[:],
                                     start=True, stop=True)
                    ind = sm.tile([128, NSEG], f32, tag="ind")
                    nc.vector.tensor_scalar(ind[:], rpm[:], spmb[:, 0:1], None,
                                            Alu.is_equal)
                    sca = sm.tile([128, 32], f32, tag="sca")
                    s2a = sm.tile([128, 1], f32, tag="s2a")
                    nc.vector.tensor_scalar(sca[:], ptA[:], spmb[:, 0:1], None,
                                            Alu.is_lt, Alu.add, accum_out=s2a[:])
                    scb = sm.tile([128, 32], f32, tag="scb")
                    s2b = sm.tile([128, 1], f32, tag="s2b")
                    nc.vector.tensor_scalar(scb[:], ptA[:], spmb[:, 0:1], None,
                                            Alu.is_gt, Alu.add, accum_out=s2b[:])
                    s2d = sm.tile([128, 1], f32, tag="s2d")
                    nc.vector.tensor_tensor(s2d[:], s2a[:], s2b[:], Alu.subtract)
                    s2bc = ps.tile([128, 1], f32, tag="s2bc")
                    nc.tensor.matmul(s2bc[:], ones_sq[:], s2d[:],
                                     start=True, stop=True)
                    # b1v_true = BETA*(1-B1)*S2*spm*ind ; b1s = b1v_true*B1^{-t}
                    b1t = sm.tile([128, NSEG], f32, tag="b1t")
                    nc.vector.tensor_scalar(b1t[:], ind[:], spmb[:, 0:1], None,
                                            Alu.mult)
                    b1v = sm.tile([128, NSEG], f32, tag="b1v")
                    nc.vector.tensor_scalar(b1v[:], b1t[:], s2bc[:, 0:1],
                                            BETA * (1.0 - B1),
                                            Alu.mult, Alu.mult)
                    b1s = sm.tile([128, NSEG], f32, tag="b1s")
                    nc.vector.tensor_scalar(b1s[:], b1v[:], SCG, None, Alu.mult)
                    # rwT = rs * (SC1*sg1 - b1s),  SC1 = (1-B1)*B1^{-t}
                    wv = sm.tile([128, NSEG], f32, tag="wv")
                    nc.vector.scalar_tensor_tensor(wv[:], sg1[:],
                                                   (1.0 - B1) * SCG,
                                                   b1s[:], Alu.mult, Alu.subtract)
                    rwT = sm.tile([128, NSEG], f32, tag="rwT")
                    nc.vector.tensor_tensor(rwT[:], wv[:], rs[:], Alu.mult)

                    # bulk gradient (bf16): gi = e*rwT + m0b
                    #   m0b = is_eq(u, rmu_m)*b1s  [TS 2x]
                    #   ew  = e * rwT              [TS 4x]
                    #   gi  = ew + m0b             [TT 2x]
                    for s in range(NSEG):
                        nc.vector.tensor_scalar(
                            m03[:, s], u3[:, s], rmu_m[:, s:s + 1],
                            b1s[:, s:s + 1], Alu.is_equal, Alu.mult)
                    for s in range(NSEG):
                        nc.vector.tensor_scalar(
                            ew3[:, s], e3[:, s], rwT[:, s:s + 1], None,
                            Alu.mult)
                    # AdamW tail, two pipelined halves
                    for h in range(2):
                        nc.vector.tensor_tensor(gi2[:, h], ew2[:, h],
                                                m0b2[:, h], Alu.add)
                        nc.vector.tensor_tensor(mt2[:, h], mt2[:, h],
                                                gi2[:, h], Alu.add)
                        nc.scalar.activation(gsq2[:, h], gi2[:, h], Act.Square,
                                             scale=SQC)
                        nc.gpsimd.tensor_tensor(v2[:, h], v2[:, h],
                                                gsq2[:, h], Alu.add)
                        # lnv into gsq (dead), rdn into m0b (dead)
                        nc.scalar.activation(gsq2[:, h], v2[:, h], Act.Ln,
                                             bias=b37)
                        nc.scalar.activation(m0b2[:, h], gsq2[:, h], Act.Exp,
                                             scale=-0.5, bias=lncb)
                        # t3 into ew (dead)
                        nc.vector.tensor_tensor(ew2[:, h], mt2[:, h],
                                                m0b2[:, h], Alu.mult)
                        nc.vector.tensor_tensor(u2[:, h], u2[:, h],
                                                ew2[:, h], Alu.subtract)

                    # side state (exact y-column trajectory, true space)
                    p_y = sm.tile([128, NSEG], f32, tag="p_y")
                    nc.vector.tensor_tensor(p_y[:], e_y[:], rs[:], Alu.mult)
                    ty1 = sm.tile([128, NSEG], f32, tag="ty1")
                    nc.vector.tensor_scalar(ty1[:], p_y[:], 1.0, None,
                                            Alu.subtract)
                    ty2 = sm.tile([128, NSEG], f32, tag="ty2")
                    nc.vector.tensor_tensor(ty2[:], ty1[:], sg1[:], Alu.mult)
                    ty3 = sm.tile([128, NSEG], f32, tag="ty3")
                    nc.vector.tensor_scalar(ty3[:], wm[:], -1.0, 1.0,
                                            Alu.mult, Alu.add)
                    ty4 = sm.tile([128, NSEG], f32, tag="ty4")
                    nc.vector.tensor_tensor(ty4[:], ty3[:], p_y[:], Alu.subtract)
                    ty5 = sm.tile([128, NSEG], f32, tag="ty5")
                    nc.vector.tensor_tensor(ty5[:], ty4[:], b1v[:], Alu.mult)
                    gyi = sm.tile([128, NSEG], f32, tag="gyi")
                    nc.vector.scalar_tensor_tensor(gyi[:], ty2[:], 1.0 - B1,
                                                   ty5[:], Alu.mult, Alu.add)
                    nc.vector.scalar_tensor_tensor(m_y[:], m_y[:], B1, gyi[:],
                                                   Alu.mult, Alu.add)
                    gys = sm.tile([128, NSEG], f32, tag="gys")
                    KSQ = math.sqrt(1.0 - B2) / (1.0 - B1)
                    nc.scalar.activation(gys[:], gyi[:], Act.Square, scale=KSQ)
                    nc.vector.scalar_tensor_tensor(v_y[:], v_y[:], B2, gys[:],
                                                   Alu.mult, Alu.add)
                    lny = sm.tile([128, NSEG], f32, tag="lny")
                    nc.scalar.activation(lny[:], v_y[:], Act.Ln, bias=b37)
                    sqy = sm.tile([128, NSEG], f32, tag="sqy")
                    nc.scalar.activation(sqy[:], lny[:], Act.Exp, scale=0.5,
                                         bias=sqb)
                    dny = sm.tile([128, NSEG], f32, tag="dny")
                    nc.vector.tensor_scalar(dny[:], sqy[:], EPS, ilr,
                                            Alu.add, Alu.mult)
                    rdy = sm.tile([128, NSEG], f32, tag="rdy")
                    nc.vector.reciprocal_approx_fast(rdy[:], dny[:])
                    t3y = sm.tile([128, NSEG], f32, tag="t3y")
                    nc.vector.tensor_tensor(t3y[:], m_y[:], rdy[:], Alu.mult)
                    nc.vector.scalar_tensor_tensor(u_y[:], u_y[:], WDF, t3y[:],
                                                   Alu.mult, Alu.subtract)
                    # e_y for next iteration (off critical path)
                    nc.scalar.activation(e_y[:], u_y[:], Act.Exp)

                # ---------------- merge y column back, write out ----------
                # u_true = WDF^KAPPA * utl ; fold into the (1-ohm) factor.
                WFK = float(WDF ** KAPPA)
                ohm2f = gsq
                nc.vector.tensor_scalar(ohm2f[:], ohm[:], -WFK, WFK,
                                        Alu.mult, Alu.add)
                for s in range(NSEG):
                    nc.vector.tensor_scalar(ohm3[:, s], ohm3[:, s],
                                            u_y[:, s:s + 1], None, Alu.mult)
                ub = v
                nc.vector.tensor_tensor(ub[:], u[:], ohm2f[:], Alu.mult)
                nc.vector.tensor_tensor(ub[:], ub[:], ohm[:], Alu.add)
                nc.sync.dma_start(
                    out_d[:].rearrange("(s p) c -> p s c", p=128),
                    ub[:].rearrange("p (s c) -> p s c", s=NSEG),
                )

    nc.compile()
    return nc


def _get_nc():
    if "nc" not in _CACHE:
        _CACHE["nc"] = _build()
    return _CACHE["nc"]


def _make_in_maps(x, W, b):
    x = np.ascontiguousarray(np.asarray(x, np.float32))
    W = np.ascontiguousarray(np.asarray(W, np.float32))
    b = np.ascontiguousarray(np.asarray(b, np.float32)).reshape(1, C)
    eye = np.eye(128, dtype=np.float32)

    cc = _iter_consts()
    cst = np.zeros((128, 3 * KAPPA + 4), np.float32)
    cst[:, 0] = 1e-37
    cst[:, 4:4 + KAPPA] = np.float32(cc["sqv_bias"])[None, :]
    cst[:, 4 + KAPPA:4 + 2 * KAPPA] = np.float32(EPS / cc["lr_t"])[None, :]
    cst[:, 4 + 2 * KAPPA:4 + 3 * KAPPA] = np.float32(cc["lnc"])[None, :]

    in_maps = []
    for c in range(NCORES):
        xs = x[c * BL:(c + 1) * BL]
        in_maps.append({
            "xT": np.ascontiguousarray(xs.T),
            "W": W, "b": b, "eye": eye, "cst": cst,
        })
    return in_maps


def _run(x, W, b, trace=False):
    nc = _get_nc()
    in_maps = _make_in_maps(x, W, b)
    res = run_bass_kernel_spmd(nc, in_maps, core_ids=list(range(NCORES)),
                               trace=trace)
    out = np.concatenate([res.results[c]["out"] for c in range(NCORES)], axis=0)
    return out, res


def kernel(**inputs):
    out, _ = _run(inputs["x"], inputs["W"], inputs["b"])
    return out
